# revision 31
# baseline (speedup 1.0000x reference)
"""PointNet-style conv (sample+knn+gather+MLP+maxpool) on 8 NeuronCores.

Data-parallel over the B=8 point clouds: core c owns cloud c end-to-end
(knn, gather, per-edge MLP, max-pool are all cloud-local).

Per-core device pipeline:
  gT  = ([x; pos; 1] @ [W1; b1])^T            # [128, 4096] layer-1 point feats
  wT  = W1[64:67]^T-matmul pos_s^T            # [128, 1024] per-center offset
  S   = 2*pos_s . pos_n - |pos_n|^2           # [1024, 4096] negated-d2 + const
  top-32 per center via chunked vector.max (top-8 of 16 chunks of 256,
    then 4 extraction rounds on the 128 candidates) -> threshold tau ->
    indices extracted by max-ing a masked negated-iota; rows where the
    chunk cap could have lost a true neighbour are flagged and patched
    exactly on the host (expected ~2% of rows).
  edge MLP: h1 = relu(gT[:, j] - wT[:, i]) (fp16), y = W2^T-matmul h1 on PE,
    max over the 32 neighbours via a pairwise-max tree, then out = relu(max+b2).
"""

import numpy as np

import concourse.bass as bass
import concourse.mybir as mybir
import concourse.tile as tile
from concourse.bass_utils import run_bass_kernel_spmd
from concourse.masks import make_identity

B = 8
NP = 4096
STRIDE = 4
NS = NP // STRIDE     # 1024
K = 32
DIN = 64
H1 = 128
H2 = 256

SEL_CHUNK = 256       # selection chunk size (top-8 kept per chunk)
N_SEL_CHUNKS = NP // SEL_CHUNK     # 16
EDGE_CHUNK = 2048     # edges per gather/MLP chunk
N_EDGE_CHUNKS = NS * K // EDGE_CHUNK   # 8
NEG_FILL = -3.0e38

USE_F32R = False

f32 = mybir.dt.float32
f32r = mybir.dt.float32r
f16 = mybir.dt.float16
i16 = mybir.dt.int16
u16 = mybir.dt.uint16


def _mmdt(ap):
    return ap.bitcast(f32r) if USE_F32R else ap


def build_kernel():
    from concourse import bacc
    nc = bacc.Bacc()
    xpt = nc.declare_dram_parameter("xpt", [68, NP], f32, isOutput=False)
    w1r = nc.declare_dram_parameter("w1r", [68, H1], f32, isOutput=False)
    w2 = nc.declare_dram_parameter("w2", [H1, H2], f16, isOutput=False)
    b2c = nc.declare_dram_parameter("b2c", [128, 2], f32, isOutput=False)
    yt = nc.declare_dram_parameter("yt", [H2, NS], f32, isOutput=True)
    flags = nc.declare_dram_parameter("flags", [128, 8], f32, isOutput=True)

    with tile.TileContext(nc) as tc:
        _build(nc, tc, xpt, w1r, w2, b2c, yt, flags)
    nc.compile()
    return nc


def _build(nc, tc, xpt, w1r, w2, b2c, yt, flags):
    import contextlib

    ctx = contextlib.ExitStack()
    with ctx:
        cst = ctx.enter_context(tc.tile_pool(name="cst", bufs=1))
        ps_s = ctx.enter_context(tc.tile_pool(name="ps_s", bufs=1, space="PSUM"))
        ps_y = ctx.enter_context(tc.tile_pool(name="ps_y", bufs=2, space="PSUM"))
        ps_m = ctx.enter_context(tc.tile_pool(name="ps_m", bufs=2, space="PSUM"))
        p_S = ctx.enter_context(tc.tile_pool(name="p_S", bufs=2))
        p_sel = ctx.enter_context(tc.tile_pool(name="p_sel", bufs=2))
        p_ga = ctx.enter_context(tc.tile_pool(name="p_ga", bufs=2))
        p_h1 = ctx.enter_context(tc.tile_pool(name="p_h1", bufs=2))
        p_yc = ctx.enter_context(tc.tile_pool(name="p_yc", bufs=2))

        # ---------------- load constants ----------------
        # xpt and sq are only needed during precompute; borrow the score pool
        sb_xpt = p_S.tile([68, NP], f32, tag="S")
        sb_w1r = cst.tile([68, H1], f32)
        sb_w2 = cst.tile([H1, H2], f16)
        sb_b2 = cst.tile([128, 2], f32)
        nc.sync.dma_start(out=sb_xpt[64:68, :], in_=xpt[64:68, :])
        nc.sync.dma_start(out=sb_xpt[0:64, :], in_=xpt[0:64, :])
        nc.sync.dma_start(out=sb_w1r[:, :], in_=w1r[:, :])
        nc.sync.dma_start(out=sb_w2[:, :], in_=w2[:, :])
        nc.sync.dma_start(out=sb_b2[:, :], in_=b2c[:, :])

        sb_id = cst.tile([128, 128], f32)
        make_identity(nc, sb_id[:, :])

        # ---------------- precompute score rhs/lhs first (gates selection) ----
        posT = sb_xpt[64:67, :]

        # score rhs [4, NP]: rows 0-2 = pos^T, row 3 = |pos|^2.
        # |pos|^2 is matmul'd straight onto psum partition 3 (lhsT has ones
        # only in column 3) so no partition-3 engine write is ever needed:
        # copy psum rows 0-3 (garbage+n2) first, then overwrite 0-2 with pos.
        sb_r4 = cst.tile([4, NP], f32)
        sb_sq = p_S.tile([3, NP], f32, tag="S")
        nc.scalar.square(sb_sq[:, :], posT)
        sb_o34 = cst.tile([3, 4], f32)
        nc.vector.memset(sb_o34[:, :], 0.0)
        nc.vector.memset(sb_o34[:, 3:4], 1.0)
        for j in range(NP // 512):
            pn = ps_m.tile([128, 512], f32, tag="ps_m")
            nc.tensor.matmul(pn[0:4, :], lhsT=_mmdt(sb_o34[:, :]),
                             rhs=_mmdt(sb_sq[:, j * 512:(j + 1) * 512]),
                             start=True, stop=True)
            nc.scalar.copy(sb_r4[0:4, j * 512:(j + 1) * 512], pn[0:4, :])
        nc.vector.tensor_copy(sb_r4[0:3, :], posT)

        # score lhsT [4, NS]: rows 0-2 = 2*pos_s^T, row 3 = -1
        sb_scl = cst.tile([4, NS], f32)
        nc.vector.memset(sb_scl[:, :], -1.0)
        nc.scalar.mul(sb_scl[0:3, :], sb_xpt[64:67, ::STRIDE], 2.0)

        # wT [128, NS] = (pos_s @ W1[64:67])^T
        sb_wT = cst.tile([H1, NS], f32)
        for j in range(NS // 512):
            pw = ps_m.tile([128, 512], f32, tag="ps_m")
            nc.tensor.matmul(pw[:, :], lhsT=_mmdt(sb_w1r[64:67, :]),
                             rhs=_mmdt(sb_xpt[64:67, j * 2048:(j + 1) * 2048:STRIDE]),
                             start=True, stop=True)
            nc.scalar.copy(sb_wT[:, j * 512:(j + 1) * 512], pw[:, :])

        # gT [128, NP] = ([x;pos;1] @ [W1;b1])^T, fp32 (only needed by gathers)
        sb_gT = cst.tile([H1, NP], f32)
        for j in range(NP // 512):
            pg = ps_m.tile([128, 512], f32, tag="ps_m")
            nc.tensor.matmul(pg[:, :], lhsT=_mmdt(sb_w1r[:, :]),
                             rhs=_mmdt(sb_xpt[:, j * 512:(j + 1) * 512]),
                             start=True, stop=True)
            nc.scalar.copy(sb_gT[:, j * 512:(j + 1) * 512], pg[:, :])

        # edge index list (int16, 16-partition wrapped, replicated to 128)
        sb_EI = cst.tile([128, NS * K // 16], i16)
        sb_FL = cst.tile([128, 8], f32)

        # chunk-base iota for global index reconstruction
        sb_CB = cst.tile([128, N_SEL_CHUNKS * 8], u16)
        nc.gpsimd.iota(sb_CB[:, :], pattern=[[SEL_CHUNK, N_SEL_CHUNKS], [0, 8]],
                       base=0, channel_multiplier=0)

        # ---------------- gather + edge MLP + maxpool (interleaved) ----------
        sb_MXB = cst.tile([128, 2 * NS], f16)

        def mlp_chunk(ec):
            tail = ec >= N_EDGE_CHUNKS - 2   # DVE is idle in the drain tail
            ve = nc.vector if tail else nc.gpsimd
            ncent = EDGE_CHUNK // K  # centers per chunk
            sb_GA = p_ga.tile([128, EDGE_CHUNK], f32, tag="GA")
            nc.gpsimd.ap_gather(
                out_ap=sb_GA[:, :], in_ap=sb_gT[:, :],
                idxs_ap=sb_EI[:, ec * (EDGE_CHUNK // 16):(ec + 1) * (EDGE_CHUNK // 16)],
                channels=128, num_elems=NP, d=1, num_idxs=EDGE_CHUNK)

            sb_H1 = p_h1.tile([128, EDGE_CHUNK], f16, tag="H1")
            wslice = sb_wT[:, ec * ncent:(ec + 1) * ncent]
            ve.tensor_sub(
                sb_H1[:, :].rearrange("p (c k) -> p c k", k=K),
                sb_GA[:, :].rearrange("p (c k) -> p c k", k=K),
                wslice.to_broadcast([128, ncent, K]))
            nc.vector.tensor_scalar_max(sb_H1[:, :], sb_H1[:, :], 0.0)

            sb_Y0 = p_yc.tile([128, EDGE_CHUNK], f16, tag="Y0")
            sb_Y1 = p_yc.tile([128, EDGE_CHUNK], f16, tag="Y1")
            for ps in range(EDGE_CHUNK // 1024):
                for ft in range(2):
                    py = ps_y.tile([128, 1024], f32, tag="ps_y")
                    for h in range(2):
                        nc.tensor.matmul(
                            py[:, h * 512:(h + 1) * 512],
                            lhsT=sb_w2[:, ft * 128:(ft + 1) * 128],
                            rhs=sb_H1[:, ps * 1024 + h * 512: ps * 1024 + (h + 1) * 512],
                            start=True, stop=True)
                    dst = (sb_Y0 if ft == 0 else sb_Y1)
                    nc.scalar.copy(dst[:, ps * 1024:(ps + 1) * 1024], py[:, :])

            # pairwise-max tree, both feature halves batched per round
            sb_P1 = p_sel.tile([128, 2, ncent, 16], f16, tag="P1")
            for ft in range(2):
                src = (sb_Y0 if ft == 0 else sb_Y1)[:, :].rearrange(
                    "p (c k) -> p c k", k=K)
                nc.vector.tensor_max(sb_P1[:, ft, :, :], src[:, :, 0:16],
                                     src[:, :, 16:32])
            sb_P2 = p_sel.tile([128, 2, ncent, 8], f16, tag="P2")
            nc.vector.tensor_max(sb_P2[:, :, :, :], sb_P1[:, :, :, 0:8],
                                 sb_P1[:, :, :, 8:16])
            sb_P3 = p_sel.tile([128, 2, ncent, 4], f16, tag="P3")
            nc.vector.tensor_max(sb_P3[:, :, :, :], sb_P2[:, :, :, 0:4],
                                 sb_P2[:, :, :, 4:8])
            sb_P4 = p_sel.tile([128, 2, ncent, 2], f16, tag="P4")
            nc.vector.tensor_max(sb_P4[:, :, :, :], sb_P3[:, :, :, 0:2],
                                 sb_P3[:, :, :, 2:4])
            mxv = sb_MXB[:, :].rearrange("p (f c) -> p f c", f=2)
            nc.vector.tensor_max(
                mxv[:, :, ec * ncent:(ec + 1) * ncent, None],
                sb_P4[:, :, :, 0:1], sb_P4[:, :, :, 1:2])

        # ---------------- per-center-tile: score + top-32 selection ----------------
        for mt in range(NS // 128):
            sb_S = p_S.tile([128, NP], f32, tag="S")
            for j in range(NP // 1024):
                pscore = ps_s.tile([128, 1024], f32, tag="score")
                for h in range(2):
                    nc.tensor.matmul(
                        pscore[:, h * 512:(h + 1) * 512],
                        lhsT=_mmdt(sb_scl[:, mt * 128:(mt + 1) * 128]),
                        rhs=_mmdt(sb_r4[:, j * 1024 + h * 512: j * 1024 + (h + 1) * 512]),
                        start=True, stop=True)
                nc.scalar.copy(sb_S[:, j * 1024:(j + 1) * 1024], pscore[:, :])

            sb_C = p_sel.tile([128, 128], f32, tag="C")
            sb_L = p_sel.tile([128, 128], u16, tag="L")
            for c in range(N_SEL_CHUNKS):
                nc.vector.max(out=sb_C[:, c * 8:(c + 1) * 8],
                              in_=sb_S[:, c * SEL_CHUNK:(c + 1) * SEL_CHUNK])
            for c in range(N_SEL_CHUNKS):
                nc.vector.max_index(out=sb_L[:, c * 8:(c + 1) * 8],
                                    in_max=sb_C[:, c * 8:(c + 1) * 8],
                                    in_values=sb_S[:, c * SEL_CHUNK:(c + 1) * SEL_CHUNK])

            # 4 extraction rounds on the candidate array -> tau (32nd largest)
            sb_V = p_sel.tile([128, 32], f32, tag="V")
            sb_D = p_sel.tile([128, 128], f32, tag="D")
            nc.vector.max(out=sb_V[:, 0:8], in_=sb_C[:, :])
            nc.vector.match_replace(out=sb_D[:, :], in_to_replace=sb_V[:, 0:8],
                                    in_values=sb_C[:, :], imm_value=NEG_FILL)
            for r in range(1, 4):
                nc.vector.max(out=sb_V[:, r * 8:(r + 1) * 8], in_=sb_D[:, :])
                nc.vector.match_replace(out=sb_D[:, :],
                                        in_to_replace=sb_V[:, r * 8:(r + 1) * 8],
                                        in_values=sb_D[:, :], imm_value=NEG_FILL)
            tau = sb_V[:, 31:32]
            # rank 33-40, for the near-tie flag
            sb_V5 = p_sel.tile([128, 8], f32, tag="V5")
            nc.vector.max(out=sb_V5[:, :], in_=sb_D[:, :])

            # mask of candidates >= tau; count & chunk-full flags
            sb_M = p_sel.tile([128, 128], mybir.dt.uint8, tag="M")
            nc.gpsimd.tensor_scalar(sb_M[:, :], sb_C[:, :], tau, None,
                                    op0=mybir.AluOpType.is_ge)
            sb_cnt = p_sel.tile([128, 1], f32, tag="cnt")
            nc.vector.reduce_sum(sb_cnt[:, :], sb_M[:, :], axis=mybir.AxisListType.X)
            sb_cf = p_sel.tile([128, 1], f32, tag="cf")
            nc.vector.reduce_max(sb_cf[:, :], sb_M[:, 7:128:8], axis=mybir.AxisListType.X)
            sb_ce = p_sel.tile([128, 1], f32, tag="ce")
            nc.vector.tensor_scalar(sb_ce[:, :], sb_cnt[:, :], 32.0, None,
                                    op0=mybir.AluOpType.not_equal)
            # near-tie at the 32/33 boundary: HW-vs-host fp32 rounding can
            # flip the rank; patch those rows on the host
            sb_gp = p_sel.tile([128, 1], f32, tag="gp")
            nc.vector.tensor_sub(sb_gp[:, :], tau, sb_V5[:, 0:1])
            nc.vector.tensor_scalar(sb_gp[:, :], sb_gp[:, :], 2e-5, None,
                                    op0=mybir.AluOpType.is_le)
            nc.vector.tensor_max(sb_ce[:, :], sb_ce[:, :], sb_gp[:, :])
            nc.vector.tensor_max(sb_FL[:, mt:mt + 1], sb_cf[:, :], sb_ce[:, :])

            # global candidate indices g, shifted-negated (8192-g, exact in
            # fp32); masked to 0 where below tau; extract 32 largest = 32
            # smallest kept indices
            sb_G = p_sel.tile([128, 128], u16, tag="G")
            nc.vector.tensor_add(sb_G[:, :], sb_L[:, :], sb_CB[:, :])
            sb_Gf = p_sel.tile([128, 128], f32, tag="Gf")
            nc.gpsimd.tensor_scalar(sb_Gf[:, :], sb_G[:, :], -1.0, 8192.0,
                                    op0=mybir.AluOpType.mult,
                                    op1=mybir.AluOpType.add)
            sb_Z = p_sel.tile([128, 128], f32, tag="Z")
            nc.vector.scalar_tensor_tensor(sb_Z[:, :], sb_C[:, :], tau, sb_Gf[:, :],
                                           op0=mybir.AluOpType.is_ge,
                                           op1=mybir.AluOpType.mult)

            sb_IF = p_sel.tile([128, 32], f32, tag="IF")
            for r in range(4):
                nc.vector.max(out=sb_IF[:, r * 8:(r + 1) * 8], in_=sb_Z[:, :])
                if r < 3:
                    nc.vector.match_replace(out=sb_Z[:, :],
                                            in_to_replace=sb_IF[:, r * 8:(r + 1) * 8],
                                            in_values=sb_Z[:, :], imm_value=0.0)
            # back to g, clamped into [0, 4095] (junk rows flagged & patched)
            sb_IC = p_sel.tile([128, 32], f32, tag="IC")
            nc.gpsimd.tensor_scalar(sb_IC[:, :], sb_IF[:, :], -1.0, 8192.0,
                                    op0=mybir.AluOpType.mult,
                                    op1=mybir.AluOpType.add)
            nc.gpsimd.tensor_scalar_min(sb_IC[:, :], sb_IC[:, :], 4095.0)

            # reshape [128 centers, 32] -> 16-partition-wrapped edge list block
            for h in range(2):
                pt = ps_m.tile([128, 512], f32, tag="ps_m")
                nc.tensor.transpose(pt[0:16, 0:128], sb_IC[:, h * 16:(h + 1) * 16],
                                    sb_id[:, :])
                nc.vector.tensor_copy(
                    sb_EI[0:16, mt * 256 + h: mt * 256 + 256: 2], pt[0:16, 0:128])
            # replicate this tile's slice to all 8 gpsimd 16-partition groups,
            # then immediately gather+MLP its 128 centers (overlaps with the
            # next tile's selection on DVE)
            for g in range(1, 8):
                nc.sync.dma_start(
                    out=sb_EI[g * 16:(g + 1) * 16, mt * 256:(mt + 1) * 256],
                    in_=sb_EI[0:16, mt * 256:(mt + 1) * 256])
            mlp_chunk(2 * mt)
            mlp_chunk(2 * mt + 1)

        # final bias + relu, f32 out, transposed store
        for ft in range(2):
            sb_XF = p_sel.tile([128, NS], f32, tag="XF")
            nc.scalar.activation(sb_XF[:, :], sb_MXB[:, ft * NS:(ft + 1) * NS],
                                 mybir.ActivationFunctionType.Relu,
                                 bias=sb_b2[:, ft:ft + 1], scale=1.0)
            nc.sync.dma_start(out=yt[ft * 128:(ft + 1) * 128, :], in_=sb_XF[:, :])

        nc.sync.dma_start(out=flags[:, :], in_=sb_FL[:, :])


# ---------------------------------------------------------------------------
#  host side
# ---------------------------------------------------------------------------

_NC_CACHE = {}


def _get_nc():
    if "nc" not in _NC_CACHE:
        _NC_CACHE["nc"] = build_kernel()
    return _NC_CACHE["nc"]


def _ref_d2(pos):
    """d2 for all clouds, bit-identical to the reference's jax fp32 compute."""
    import jax.numpy as jnp
    pos_b = jnp.asarray(pos, dtype=jnp.float32).reshape(B, NP, 3)
    idx_local = jnp.arange(0, NP, STRIDE)
    pos_s = pos_b[:, idx_local]
    d2 = ((pos_s ** 2).sum(-1)[:, :, None]
          + (pos_b ** 2).sum(-1)[:, None, :]
          - 2.0 * jnp.einsum('bsd,bnd->bsn', pos_s, pos_b))
    return d2


def _host_rows(x_c, pos_c, d2_rows, W1, b1, W2, b2, rows):
    """Exact reference conv for the given center rows of one cloud."""
    import jax
    ps = pos_c[::STRIDE][rows]                      # [R, 3]
    _, nbr = jax.lax.top_k(-d2_rows, K)             # [R, K], reference tie-break
    nbr = np.asarray(nbr)
    xj = x_c[nbr]                                   # [R, K, 64]
    rel = pos_c[nbr] - ps[:, None, :]
    feat = np.concatenate([xj, rel], axis=-1)
    h = np.maximum(feat @ W1 + b1, 0.0)
    h = np.maximum(h @ W2 + b2, 0.0)
    return h.max(axis=1)                            # [R, 256]


def kernel(x, pos, batch, W1, b1, W2, b2):
    x = np.asarray(x)
    pos = np.asarray(pos)
    batch_np = np.asarray(batch)
    W1 = np.asarray(W1, dtype=np.float32)
    b1 = np.asarray(b1, dtype=np.float32)
    W2 = np.asarray(W2, dtype=np.float32)
    b2 = np.asarray(b2, dtype=np.float32)

    w1r = np.concatenate([W1, b1[None, :]], axis=0).astype(np.float32)  # [68,128]
    w2h = W2.astype(np.float16)
    b2c = np.zeros((128, 2), np.float32)
    b2c[:, 0] = b2[:128]
    b2c[:, 1] = b2[128:]

    in_maps = []
    for c in range(B):
        x_c = x[c * NP:(c + 1) * NP].astype(np.float32)
        pos_c = pos[c * NP:(c + 1) * NP].astype(np.float32)
        xpt = np.empty((68, NP), np.float32)
        xpt[0:64] = x_c.T
        xpt[64:67] = pos_c.T
        xpt[67] = 1.0
        in_maps.append({"xpt": xpt, "w1r": w1r, "w2": w2h, "b2c": b2c})

    nc = _get_nc()
    res = run_bass_kernel_spmd(nc, in_maps, list(range(B))).results

    x_out = np.empty((B * NS, H2), np.float32)
    flagged = []
    for c in range(B):
        x_out[c * NS:(c + 1) * NS] = res[c]["yt"].T
        fl = res[c]["flags"]          # [128, 8]; row p, col mt -> center mt*128+p
        bad = np.argwhere(fl > 0.5)
        rows = sorted(int(mt) * 128 + int(p) for p, mt in bad)
        flagged.append(np.asarray(rows, dtype=np.int64))
    if any(r.size for r in flagged):
        d2 = np.asarray(_ref_d2(pos))   # [B, NS, NP], reference-exact fp32
        for c in range(B):
            rows = flagged[c]
            if not rows.size:
                continue
            x_c = x[c * NP:(c + 1) * NP].astype(np.float32)
            pos_c = pos[c * NP:(c + 1) * NP].astype(np.float32)
            x_out[c * NS + rows] = _host_rows(
                x_c, pos_c, d2[c][rows], W1, b1, W2, b2, rows)

    idx_local = np.arange(0, NP, STRIDE, dtype=np.int32)
    idx = (idx_local[None, :] + (np.arange(B, dtype=np.int32) * NP)[:, None]).reshape(-1)
    pos_out = np.asarray(pos).reshape(B, NP, 3)[:, idx_local].reshape(-1, 3)
    batch_out = batch_np[idx]
    return x_out, pos_out, batch_out, idx


# revision 33
# speedup vs baseline: 1.0076x; 1.0076x over previous
"""PointNet-style conv (sample+knn+gather+MLP+maxpool) on 8 NeuronCores.

Data-parallel over the B=8 point clouds: core c owns cloud c end-to-end
(knn, gather, per-edge MLP, max-pool are all cloud-local).

Per-core device pipeline:
  gT  = ([x; pos; 1] @ [W1; b1])^T            # [128, 4096] layer-1 point feats
  wT  = W1[64:67]^T-matmul pos_s^T            # [128, 1024] per-center offset
  S   = 2*pos_s . pos_n - |pos_n|^2           # [1024, 4096] negated-d2 + const
  top-32 per center via chunked vector.max (top-8 of 16 chunks of 256,
    then 4 extraction rounds on the 128 candidates) -> threshold tau ->
    indices extracted by max-ing a masked negated-iota; rows where the
    chunk cap could have lost a true neighbour are flagged and patched
    exactly on the host (expected ~2% of rows).
  edge MLP: h1 = relu(gT[:, j] - wT[:, i]) (fp16), y = W2^T-matmul h1 on PE,
    max over the 32 neighbours via a pairwise-max tree, then out = relu(max+b2).
"""

import numpy as np

import concourse.bass as bass
import concourse.mybir as mybir
import concourse.tile as tile
from concourse.bass_utils import run_bass_kernel_spmd
from concourse.masks import make_identity

B = 8
NP = 4096
STRIDE = 4
NS = NP // STRIDE     # 1024
K = 32
DIN = 64
H1 = 128
H2 = 256

SEL_CHUNK = 256       # selection chunk size (top-8 kept per chunk)
N_SEL_CHUNKS = NP // SEL_CHUNK     # 16
EDGE_CHUNK = 4096     # edges per gather/MLP chunk
N_EDGE_CHUNKS = NS * K // EDGE_CHUNK   # 8
NEG_FILL = -3.0e38

USE_F32R = False

f32 = mybir.dt.float32
f32r = mybir.dt.float32r
f16 = mybir.dt.float16
i16 = mybir.dt.int16
u16 = mybir.dt.uint16


def _mmdt(ap):
    return ap.bitcast(f32r) if USE_F32R else ap


def build_kernel():
    from concourse import bacc
    nc = bacc.Bacc()
    xpt = nc.declare_dram_parameter("xpt", [68, NP], f32, isOutput=False)
    w1r = nc.declare_dram_parameter("w1r", [68, H1], f32, isOutput=False)
    w2 = nc.declare_dram_parameter("w2", [H1, H2], f16, isOutput=False)
    b2c = nc.declare_dram_parameter("b2c", [128, 2], f32, isOutput=False)
    yt = nc.declare_dram_parameter("yt", [H2, NS], f32, isOutput=True)
    flags = nc.declare_dram_parameter("flags", [128, 8], f32, isOutput=True)

    with tile.TileContext(nc) as tc:
        _build(nc, tc, xpt, w1r, w2, b2c, yt, flags)
    nc.compile()
    return nc


def _build(nc, tc, xpt, w1r, w2, b2c, yt, flags):
    import contextlib

    ctx = contextlib.ExitStack()
    with ctx:
        cst = ctx.enter_context(tc.tile_pool(name="cst", bufs=1))
        ps_s = ctx.enter_context(tc.tile_pool(name="ps_s", bufs=1, space="PSUM"))
        ps_y = ctx.enter_context(tc.tile_pool(name="ps_y", bufs=2, space="PSUM"))
        ps_m = ctx.enter_context(tc.tile_pool(name="ps_m", bufs=2, space="PSUM"))
        p_S = ctx.enter_context(tc.tile_pool(name="p_S", bufs=2))
        p_sel = ctx.enter_context(tc.tile_pool(name="p_sel", bufs=2))
        p_ga = ctx.enter_context(tc.tile_pool(name="p_ga", bufs=2))
        p_h1 = ctx.enter_context(tc.tile_pool(name="p_h1", bufs=2))
        p_yc = ctx.enter_context(tc.tile_pool(name="p_yc", bufs=2))

        # ---------------- load constants ----------------
        # xpt and sq are only needed during precompute; borrow the score pool
        sb_xpt = p_S.tile([68, NP], f32, tag="S")
        sb_w1r = cst.tile([68, H1], f32)
        sb_w2 = cst.tile([H1, H2], f16)
        sb_b2 = cst.tile([128, 2], f32)
        nc.sync.dma_start(out=sb_xpt[64:68, :], in_=xpt[64:68, :])
        nc.sync.dma_start(out=sb_xpt[0:64, :], in_=xpt[0:64, :])
        nc.sync.dma_start(out=sb_w1r[:, :], in_=w1r[:, :])
        nc.sync.dma_start(out=sb_w2[:, :], in_=w2[:, :])
        nc.sync.dma_start(out=sb_b2[:, :], in_=b2c[:, :])

        sb_id = cst.tile([128, 128], f32)
        make_identity(nc, sb_id[:, :])

        # ---------------- precompute score rhs/lhs first (gates selection) ----
        posT = sb_xpt[64:67, :]

        # score rhs [4, NP]: rows 0-2 = pos^T, row 3 = |pos|^2.
        # |pos|^2 is matmul'd straight onto psum partition 3 (lhsT has ones
        # only in column 3) so no partition-3 engine write is ever needed:
        # copy psum rows 0-3 (garbage+n2) first, then overwrite 0-2 with pos.
        sb_r4 = cst.tile([4, NP], f32)
        sb_sq = p_S.tile([3, NP], f32, tag="S")
        nc.scalar.square(sb_sq[:, :], posT)
        sb_o34 = cst.tile([3, 4], f32)
        nc.vector.memset(sb_o34[:, :], 0.0)
        nc.vector.memset(sb_o34[:, 3:4], 1.0)
        for j in range(NP // 512):
            pn = ps_m.tile([128, 512], f32, tag="ps_m")
            nc.tensor.matmul(pn[0:4, :], lhsT=_mmdt(sb_o34[:, :]),
                             rhs=_mmdt(sb_sq[:, j * 512:(j + 1) * 512]),
                             start=True, stop=True)
            nc.scalar.copy(sb_r4[0:4, j * 512:(j + 1) * 512], pn[0:4, :])
        nc.vector.tensor_copy(sb_r4[0:3, :], posT)

        # score lhsT [4, NS]: rows 0-2 = 2*pos_s^T, row 3 = -1
        sb_scl = cst.tile([4, NS], f32)
        nc.vector.memset(sb_scl[:, :], -1.0)
        nc.scalar.mul(sb_scl[0:3, :], sb_xpt[64:67, ::STRIDE], 2.0)

        # wT [128, NS] = (pos_s @ W1[64:67])^T
        sb_wT = cst.tile([H1, NS], f32)
        for j in range(NS // 512):
            pw = ps_m.tile([128, 512], f32, tag="ps_m")
            nc.tensor.matmul(pw[:, :], lhsT=_mmdt(sb_w1r[64:67, :]),
                             rhs=_mmdt(sb_xpt[64:67, j * 2048:(j + 1) * 2048:STRIDE]),
                             start=True, stop=True)
            nc.scalar.copy(sb_wT[:, j * 512:(j + 1) * 512], pw[:, :])

        # gT [128, NP] = ([x;pos;1] @ [W1;b1])^T, fp32 (only needed by gathers)
        sb_gT = cst.tile([H1, NP], f32)
        for j in range(NP // 512):
            pg = ps_m.tile([128, 512], f32, tag="ps_m")
            nc.tensor.matmul(pg[:, :], lhsT=_mmdt(sb_w1r[:, :]),
                             rhs=_mmdt(sb_xpt[:, j * 512:(j + 1) * 512]),
                             start=True, stop=True)
            nc.scalar.copy(sb_gT[:, j * 512:(j + 1) * 512], pg[:, :])

        # edge index list (int16, 16-partition wrapped, replicated to 128)
        sb_EI = cst.tile([128, NS * K // 16], i16)
        sb_FL = cst.tile([128, 8], f32)

        # chunk-base iota for global index reconstruction
        sb_CB = cst.tile([128, N_SEL_CHUNKS * 8], u16)
        nc.gpsimd.iota(sb_CB[:, :], pattern=[[SEL_CHUNK, N_SEL_CHUNKS], [0, 8]],
                       base=0, channel_multiplier=0)

        # ---------------- gather + edge MLP + maxpool (interleaved) ----------
        sb_MXB = cst.tile([128, 2 * NS], f16)

        def mlp_chunk(ec):
            tail = ec >= N_EDGE_CHUNKS - 2   # DVE is idle in the drain tail
            ve = nc.vector if tail else nc.gpsimd
            ncent = EDGE_CHUNK // K  # centers per chunk
            sb_GA = p_ga.tile([128, EDGE_CHUNK], f32, tag="GA")
            nc.gpsimd.ap_gather(
                out_ap=sb_GA[:, :], in_ap=sb_gT[:, :],
                idxs_ap=sb_EI[:, ec * (EDGE_CHUNK // 16):(ec + 1) * (EDGE_CHUNK // 16)],
                channels=128, num_elems=NP, d=1, num_idxs=EDGE_CHUNK)

            sb_H1 = p_h1.tile([128, EDGE_CHUNK], f16, tag="H1")
            wslice = sb_wT[:, ec * ncent:(ec + 1) * ncent]
            ve.tensor_sub(
                sb_H1[:, :].rearrange("p (c k) -> p c k", k=K),
                sb_GA[:, :].rearrange("p (c k) -> p c k", k=K),
                wslice.to_broadcast([128, ncent, K]))
            nc.vector.tensor_scalar_max(sb_H1[:, :], sb_H1[:, :], 0.0)

            sb_Y0 = p_yc.tile([128, EDGE_CHUNK], f16, tag="Y0")
            sb_Y1 = p_yc.tile([128, EDGE_CHUNK], f16, tag="Y1")
            for ps in range(EDGE_CHUNK // 1024):
                for ft in range(2):
                    py = ps_y.tile([128, 1024], f32, tag="ps_y")
                    for h in range(2):
                        nc.tensor.matmul(
                            py[:, h * 512:(h + 1) * 512],
                            lhsT=sb_w2[:, ft * 128:(ft + 1) * 128],
                            rhs=sb_H1[:, ps * 1024 + h * 512: ps * 1024 + (h + 1) * 512],
                            start=True, stop=True)
                    dst = (sb_Y0 if ft == 0 else sb_Y1)
                    nc.scalar.copy(dst[:, ps * 1024:(ps + 1) * 1024], py[:, :])

            # pairwise-max tree, both feature halves batched per round
            sb_P1 = p_sel.tile([128, 2, ncent, 16], f16, tag="P1")
            for ft in range(2):
                src = (sb_Y0 if ft == 0 else sb_Y1)[:, :].rearrange(
                    "p (c k) -> p c k", k=K)
                nc.vector.tensor_max(sb_P1[:, ft, :, :], src[:, :, 0:16],
                                     src[:, :, 16:32])
            sb_P2 = p_sel.tile([128, 2, ncent, 8], f16, tag="P2")
            nc.vector.tensor_max(sb_P2[:, :, :, :], sb_P1[:, :, :, 0:8],
                                 sb_P1[:, :, :, 8:16])
            sb_P3 = p_sel.tile([128, 2, ncent, 4], f16, tag="P3")
            nc.vector.tensor_max(sb_P3[:, :, :, :], sb_P2[:, :, :, 0:4],
                                 sb_P2[:, :, :, 4:8])
            sb_P4 = p_sel.tile([128, 2, ncent, 2], f16, tag="P4")
            nc.vector.tensor_max(sb_P4[:, :, :, :], sb_P3[:, :, :, 0:2],
                                 sb_P3[:, :, :, 2:4])
            mxv = sb_MXB[:, :].rearrange("p (f c) -> p f c", f=2)
            nc.vector.tensor_max(
                mxv[:, :, ec * ncent:(ec + 1) * ncent, None],
                sb_P4[:, :, :, 0:1], sb_P4[:, :, :, 1:2])

        # ---------------- per-center-tile: score + top-32 selection ----------------
        for mt in range(NS // 128):
            sb_S = p_S.tile([128, NP], f32, tag="S")
            for j in range(NP // 1024):
                pscore = ps_s.tile([128, 1024], f32, tag="score")
                for h in range(2):
                    nc.tensor.matmul(
                        pscore[:, h * 512:(h + 1) * 512],
                        lhsT=_mmdt(sb_scl[:, mt * 128:(mt + 1) * 128]),
                        rhs=_mmdt(sb_r4[:, j * 1024 + h * 512: j * 1024 + (h + 1) * 512]),
                        start=True, stop=True)
                nc.scalar.copy(sb_S[:, j * 1024:(j + 1) * 1024], pscore[:, :])

            sb_C = p_sel.tile([128, 128], f32, tag="C")
            sb_L = p_sel.tile([128, 128], u16, tag="L")
            for c in range(N_SEL_CHUNKS):
                nc.vector.max(out=sb_C[:, c * 8:(c + 1) * 8],
                              in_=sb_S[:, c * SEL_CHUNK:(c + 1) * SEL_CHUNK])
            for c in range(N_SEL_CHUNKS):
                nc.vector.max_index(out=sb_L[:, c * 8:(c + 1) * 8],
                                    in_max=sb_C[:, c * 8:(c + 1) * 8],
                                    in_values=sb_S[:, c * SEL_CHUNK:(c + 1) * SEL_CHUNK])

            # 4 extraction rounds on the candidate array -> tau (32nd largest)
            sb_V = p_sel.tile([128, 32], f32, tag="V")
            sb_D = p_sel.tile([128, 128], f32, tag="D")
            nc.vector.max(out=sb_V[:, 0:8], in_=sb_C[:, :])
            nc.vector.match_replace(out=sb_D[:, :], in_to_replace=sb_V[:, 0:8],
                                    in_values=sb_C[:, :], imm_value=NEG_FILL)
            for r in range(1, 4):
                nc.vector.max(out=sb_V[:, r * 8:(r + 1) * 8], in_=sb_D[:, :])
                nc.vector.match_replace(out=sb_D[:, :],
                                        in_to_replace=sb_V[:, r * 8:(r + 1) * 8],
                                        in_values=sb_D[:, :], imm_value=NEG_FILL)
            tau = sb_V[:, 31:32]
            # rank 33-40, for the near-tie flag
            sb_V5 = p_sel.tile([128, 8], f32, tag="V5")
            nc.vector.max(out=sb_V5[:, :], in_=sb_D[:, :])

            # mask of candidates >= tau; count & chunk-full flags
            sb_M = p_sel.tile([128, 128], mybir.dt.uint8, tag="M")
            nc.gpsimd.tensor_scalar(sb_M[:, :], sb_C[:, :], tau, None,
                                    op0=mybir.AluOpType.is_ge)
            sb_cnt = p_sel.tile([128, 1], f32, tag="cnt")
            nc.vector.reduce_sum(sb_cnt[:, :], sb_M[:, :], axis=mybir.AxisListType.X)
            sb_cf = p_sel.tile([128, 1], f32, tag="cf")
            nc.vector.reduce_max(sb_cf[:, :], sb_M[:, 7:128:8], axis=mybir.AxisListType.X)
            sb_ce = p_sel.tile([128, 1], f32, tag="ce")
            nc.vector.tensor_scalar(sb_ce[:, :], sb_cnt[:, :], 32.0, None,
                                    op0=mybir.AluOpType.not_equal)
            # near-tie at the 32/33 boundary: HW-vs-host fp32 rounding can
            # flip the rank; patch those rows on the host
            sb_gp = p_sel.tile([128, 1], f32, tag="gp")
            nc.vector.tensor_sub(sb_gp[:, :], tau, sb_V5[:, 0:1])
            nc.vector.tensor_scalar(sb_gp[:, :], sb_gp[:, :], 2e-5, None,
                                    op0=mybir.AluOpType.is_le)
            nc.vector.tensor_max(sb_ce[:, :], sb_ce[:, :], sb_gp[:, :])
            nc.vector.tensor_max(sb_FL[:, mt:mt + 1], sb_cf[:, :], sb_ce[:, :])

            # global candidate indices g, shifted-negated (8192-g, exact in
            # fp32); masked to 0 where below tau; extract 32 largest = 32
            # smallest kept indices
            sb_G = p_sel.tile([128, 128], u16, tag="G")
            nc.vector.tensor_add(sb_G[:, :], sb_L[:, :], sb_CB[:, :])
            sb_Gf = p_sel.tile([128, 128], f32, tag="Gf")
            nc.gpsimd.tensor_scalar(sb_Gf[:, :], sb_G[:, :], -1.0, 8192.0,
                                    op0=mybir.AluOpType.mult,
                                    op1=mybir.AluOpType.add)
            sb_Z = p_sel.tile([128, 128], f32, tag="Z")
            nc.vector.scalar_tensor_tensor(sb_Z[:, :], sb_C[:, :], tau, sb_Gf[:, :],
                                           op0=mybir.AluOpType.is_ge,
                                           op1=mybir.AluOpType.mult)

            sb_IF = p_sel.tile([128, 32], f32, tag="IF")
            for r in range(4):
                nc.vector.max(out=sb_IF[:, r * 8:(r + 1) * 8], in_=sb_Z[:, :])
                if r < 3:
                    nc.vector.match_replace(out=sb_Z[:, :],
                                            in_to_replace=sb_IF[:, r * 8:(r + 1) * 8],
                                            in_values=sb_Z[:, :], imm_value=0.0)
            # back to g, clamped into [0, 4095] (junk rows flagged & patched)
            sb_IC = p_sel.tile([128, 32], f32, tag="IC")
            nc.gpsimd.tensor_scalar(sb_IC[:, :], sb_IF[:, :], -1.0, 8192.0,
                                    op0=mybir.AluOpType.mult,
                                    op1=mybir.AluOpType.add)
            nc.gpsimd.tensor_scalar_min(sb_IC[:, :], sb_IC[:, :], 4095.0)

            # reshape [128 centers, 32] -> 16-partition-wrapped edge list block
            for h in range(2):
                pt = ps_m.tile([128, 512], f32, tag="ps_m")
                nc.tensor.transpose(pt[0:16, 0:128], sb_IC[:, h * 16:(h + 1) * 16],
                                    sb_id[:, :])
                nc.vector.tensor_copy(
                    sb_EI[0:16, mt * 256 + h: mt * 256 + 256: 2], pt[0:16, 0:128])
            # replicate this tile's slice to all 8 gpsimd 16-partition groups,
            # then immediately gather+MLP its 128 centers (overlaps with the
            # next tile's selection on DVE)
            for g in range(1, 8):
                nc.sync.dma_start(
                    out=sb_EI[g * 16:(g + 1) * 16, mt * 256:(mt + 1) * 256],
                    in_=sb_EI[0:16, mt * 256:(mt + 1) * 256])
            mlp_chunk(mt)

        # final bias + relu, f32 out, transposed store
        for ft in range(2):
            sb_XF = p_sel.tile([128, NS], f32, tag="XF")
            nc.scalar.activation(sb_XF[:, :], sb_MXB[:, ft * NS:(ft + 1) * NS],
                                 mybir.ActivationFunctionType.Relu,
                                 bias=sb_b2[:, ft:ft + 1], scale=1.0)
            nc.sync.dma_start(out=yt[ft * 128:(ft + 1) * 128, :], in_=sb_XF[:, :])

        nc.sync.dma_start(out=flags[:, :], in_=sb_FL[:, :])


# ---------------------------------------------------------------------------
#  host side
# ---------------------------------------------------------------------------

_NC_CACHE = {}


def _get_nc():
    if "nc" not in _NC_CACHE:
        _NC_CACHE["nc"] = build_kernel()
    return _NC_CACHE["nc"]


def _ref_d2(pos):
    """d2 for all clouds, bit-identical to the reference's jax fp32 compute."""
    import jax.numpy as jnp
    pos_b = jnp.asarray(pos, dtype=jnp.float32).reshape(B, NP, 3)
    idx_local = jnp.arange(0, NP, STRIDE)
    pos_s = pos_b[:, idx_local]
    d2 = ((pos_s ** 2).sum(-1)[:, :, None]
          + (pos_b ** 2).sum(-1)[:, None, :]
          - 2.0 * jnp.einsum('bsd,bnd->bsn', pos_s, pos_b))
    return d2


def _host_rows(x_c, pos_c, d2_rows, W1, b1, W2, b2, rows):
    """Exact reference conv for the given center rows of one cloud."""
    import jax
    ps = pos_c[::STRIDE][rows]                      # [R, 3]
    _, nbr = jax.lax.top_k(-d2_rows, K)             # [R, K], reference tie-break
    nbr = np.asarray(nbr)
    xj = x_c[nbr]                                   # [R, K, 64]
    rel = pos_c[nbr] - ps[:, None, :]
    feat = np.concatenate([xj, rel], axis=-1)
    h = np.maximum(feat @ W1 + b1, 0.0)
    h = np.maximum(h @ W2 + b2, 0.0)
    return h.max(axis=1)                            # [R, 256]


def kernel(x, pos, batch, W1, b1, W2, b2):
    x = np.asarray(x)
    pos = np.asarray(pos)
    batch_np = np.asarray(batch)
    W1 = np.asarray(W1, dtype=np.float32)
    b1 = np.asarray(b1, dtype=np.float32)
    W2 = np.asarray(W2, dtype=np.float32)
    b2 = np.asarray(b2, dtype=np.float32)

    w1r = np.concatenate([W1, b1[None, :]], axis=0).astype(np.float32)  # [68,128]
    w2h = W2.astype(np.float16)
    b2c = np.zeros((128, 2), np.float32)
    b2c[:, 0] = b2[:128]
    b2c[:, 1] = b2[128:]

    in_maps = []
    for c in range(B):
        x_c = x[c * NP:(c + 1) * NP].astype(np.float32)
        pos_c = pos[c * NP:(c + 1) * NP].astype(np.float32)
        xpt = np.empty((68, NP), np.float32)
        xpt[0:64] = x_c.T
        xpt[64:67] = pos_c.T
        xpt[67] = 1.0
        in_maps.append({"xpt": xpt, "w1r": w1r, "w2": w2h, "b2c": b2c})

    nc = _get_nc()
    res = run_bass_kernel_spmd(nc, in_maps, list(range(B))).results

    x_out = np.empty((B * NS, H2), np.float32)
    flagged = []
    for c in range(B):
        x_out[c * NS:(c + 1) * NS] = res[c]["yt"].T
        fl = res[c]["flags"]          # [128, 8]; row p, col mt -> center mt*128+p
        bad = np.argwhere(fl > 0.5)
        rows = sorted(int(mt) * 128 + int(p) for p, mt in bad)
        flagged.append(np.asarray(rows, dtype=np.int64))
    if any(r.size for r in flagged):
        d2 = np.asarray(_ref_d2(pos))   # [B, NS, NP], reference-exact fp32
        for c in range(B):
            rows = flagged[c]
            if not rows.size:
                continue
            x_c = x[c * NP:(c + 1) * NP].astype(np.float32)
            pos_c = pos[c * NP:(c + 1) * NP].astype(np.float32)
            x_out[c * NS + rows] = _host_rows(
                x_c, pos_c, d2[c][rows], W1, b1, W2, b2, rows)

    idx_local = np.arange(0, NP, STRIDE, dtype=np.int32)
    idx = (idx_local[None, :] + (np.arange(B, dtype=np.int32) * NP)[:, None]).reshape(-1)
    pos_out = np.asarray(pos).reshape(B, NP, 3)[:, idx_local].reshape(-1, 3)
    batch_out = batch_np[idx]
    return x_out, pos_out, batch_out, idx
